# revision 3
# baseline (speedup 1.0000x reference)
"""Trainium2 Bass kernel for nn_BasicBlock (conv3x3-BN-perelem_act-conv3x3-BN + act shortcut).

Data-parallel over batch: 32 images -> 4 per core x 8 cores.

Layout: each 64x112x112 image is split into top/bottom 56-row halves mapped to
SBUF partitions 0-63 (top, one per channel) and 64-127 (bottom), so every
elementwise op uses all 128 lanes.

Conv3x3: 9 accumulating matmuls per 4-row block with a BLOCK-DIAGONAL
128x128 f16 stationary [[W,0],[0,W]] — one matmul computes both halves
(PE cost is per output free element, so fusing the halves halves PE time
vs. two 64x64 tile matmuls). BN scale (gamma/sqrt(var+eps)) is folded into
the conv weights per output channel; BN1 bias rides the PSUM eviction
(ACT Identity with per-partition bias); BN2 bias is folded into the K map.

Per-element activation (codes 0..3 = relu/identity/tanh/sigmoid):
  act(y) = sigmoid(s1*y) * (b*y + a) + d
with host-precomputed per-element f16 arrays
  s1 = {relu: 512, id: 0, tanh: 2, sigmoid: 1}
  b  = {relu: 1, id: 2, else 0}        (id: sigmoid(0)=0.5 -> 0.5*2y = y)
  a  = {tanh: 2, sigmoid: 1, else 0}
  d  = {tanh: -1, else 0}
The d-terms never touch the device math: the feature-layer d propagates
through conv2 linearly, so K = conv2_bnfolded(d1map) + bn2_bias + d2map is
precomputed on host and added in the final combine.
"""

import sys

sys.path.insert(0, "/opt/trn_rl_repo")

import numpy as np
from contextlib import ExitStack

import concourse.bass as bass
import concourse.bacc as bacc
import concourse.tile as tile
import concourse.mybir as mybir
from concourse.bass_utils import run_bass_kernel_spmd

F16 = np.float16
MDT = mybir.dt.float16
EPS = 1e-5
KREL = 512.0   # sigmoid(KREL*y) ~ step(y) for the relu branch

B, C, H, W = 32, 64, 112, 112
NCORES = 8
BPC = B // NCORES          # images per core
SEC = H // 2               # rows per half-section (56)
HP, WP = SEC + 2, W + 2    # padded section: 58 x 114
NU = SEC // 8              # 8-row elementwise units per half (7)

TAPS = [(ky, kx) for ky in (-1, 0, 1) for kx in (-1, 0, 1)]
MASKS_F = ["s1f", "bf", "af"]
MASKS_S = ["s1s", "bs", "as_", "kmap"]

LAST_RESULT = None  # BassKernelResults of the most recent kernel() call


def _split_halves(m):
    """[64, 112, X] -> [128, 56, X]: top rows on partitions 0-63, bottom on 64-127."""
    return np.concatenate([m[:, 0:SEC, :], m[:, SEC:H, :]], axis=0)


def _pad_split_image(img):
    """[64,112,112] fp -> [128, 58, 114] f16 padded split layout (1px halo)."""
    p = np.zeros((C, H + 2, W + 2), np.float32)
    p[:, 1:113, 1:113] = img
    top = p[:, 0:HP, :]
    bot = p[:, SEC:SEC + HP, :]
    return np.concatenate([top, bot], axis=0).astype(F16)


def _mask_arrays(codes):
    """codes [C*H*W] int32 -> (s1, b, a, d) [64,112,112] f32 arrays."""
    c = codes.reshape(C, H, W)
    s1 = np.select([c == 0, c == 1, c == 2, c == 3], [KREL, 0.0, 2.0, 1.0]).astype(np.float32)
    b = np.select([c == 0, c == 1], [1.0, 2.0], 0.0).astype(np.float32)
    a = np.select([c == 2, c == 3], [2.0, 1.0], 0.0).astype(np.float32)
    d = np.where(c == 2, -1.0, 0.0).astype(np.float32)
    return s1, b, a, d


def _host_conv3x3(x, w):
    """x [C,H,W] f32, w [O,I,3,3] f32 -> [O,H,W] f32 (pad=1)."""
    xp = np.zeros((C, H + 2, W + 2), np.float32)
    xp[:, 1:H + 1, 1:W + 1] = x
    out = np.zeros((w.shape[0], H, W), np.float32)
    for ky in range(3):
        for kx in range(3):
            out += np.einsum(
                "ihw,oi->ohw",
                xp[:, ky:ky + H, kx:kx + W],
                w[:, :, ky, kx],
                optimize=True,
            )
    return out


def _build_program():
    nc = bacc.Bacc("TRN2", target_bir_lowering=False, debug=False)

    xin = nc.dram_tensor("xin", [BPC, 128, HP, WP], MDT, kind="ExternalInput")
    # both conv weights in one DRAM tensor, SBUF-layout-major: [128, 18, 128]
    wd = nc.dram_tensor("w", [128, 18, 128], MDT, kind="ExternalInput")
    b1d = nc.dram_tensor("b1", [128, 1], mybir.dt.float32, kind="ExternalInput")
    mdram = {
        k: nc.dram_tensor(k, [128, SEC, W], MDT, kind="ExternalInput")
        for k in MASKS_F + MASKS_S
    }
    outd = nc.dram_tensor("out", [BPC, 128, SEC, W], MDT, kind="ExternalOutput")

    ID = mybir.ActivationFunctionType.Identity
    SG = mybir.ActivationFunctionType.Sigmoid

    with tile.TileContext(nc) as tc, ExitStack() as ctx:
        wp = ctx.enter_context(tc.tile_pool(name="w", bufs=1))
        mp = ctx.enter_context(tc.tile_pool(name="m", bufs=1))
        xp = ctx.enter_context(tc.tile_pool(name="x", bufs=2))
        hp = ctx.enter_context(tc.tile_pool(name="h", bufs=2))
        ep = ctx.enter_context(tc.tile_pool(name="e", bufs=2))
        pp = ctx.enter_context(tc.tile_pool(name="ps", bufs=2, space="PSUM"))

        wt = wp.tile([128, 18, 128], MDT, tag="w")
        w1t = wt[:, 0:9, :]
        w2t = wt[:, 9:18, :]
        b1t = wp.tile([128, 1], mybir.dt.float32, tag="b1")

        mt = {k: mp.tile([128, SEC, W], MDT, tag=k, name=k) for k in MASKS_F + MASKS_S}

        xts = {}
        hts = {}

        def load_x(n):
            xts[n] = xp.tile([128, HP, WP], MDT, tag="xt", name=f"xt{n}")
            nc.sync.dma_start(xts[n][:], xin[n, :, :, :])

        xts_first = xp.tile([128, HP, WP], MDT, tag="xt", name="xt_first")
        xts[0] = xts_first

        def load_masks(names):
            for k in names:
                nc.sync.dma_start(mt[k][:], mdram[k][:, :, :])

        def conv_unit(src, wt, ps, r0):
            """9-tap block-diag conv into ps[:, 0:8, 0:112] for output rows
            r0..r0+7 of each half (both halves in one matmul)."""
            for i in (0, 1):
                for t, (ky, kx) in enumerate(TAPS):
                    rs = r0 + 4 * i + 1 + ky
                    nc.tensor.matmul(
                        ps[:, 4 * i:4 * i + 4, 0:112], wt[:, t, :],
                        src[:, rs:rs + 4, kx + 1:kx + 113],
                        start=(t == 0), stop=(t == 8),
                        skip_group_check=True,
                    )

        def l1_phase(n):
            if n == 0:
                nc.sync.dma_start(wt[:, 0:1, :], wd[:, 0:1, :])
                nc.sync.dma_start(xts[0][:, 0:10, :], xin[0, :, 0:10, :])
                nc.sync.dma_start(wt[:, 1:18, :], wd[:, 1:18, :])
                nc.sync.dma_start(xts[0][:, 10:HP, :], xin[0, :, 10:HP, :])
                nc.sync.dma_start(b1t[:], b1d[:, :])
                load_masks(MASKS_F)
            elif n == 1:
                load_x(n)
                load_masks(MASKS_S)
            else:
                load_x(n)
            xt = xts[n]
            ht = hp.tile([128, HP, WP], MDT, tag="ht", name=f"ht{n}")
            hts[n] = ht
            if n < 2:
                # borders stay zero across reuses; interior is fully rewritten
                nc.gpsimd.memset(ht[:], 0.0)
            for u in range(NU):
                r0 = 8 * u
                ps = pp.tile([128, 8, 128], mybir.dt.float32, tag="ps1")
                conv_unit(xt, w1t, ps, r0)
                y16 = ep.tile([128, 8, 112], MDT, tag="y16")
                nc.scalar.activation(y16[:], ps[:, :, 0:112], ID, bias=b1t[:])
                m1 = ep.tile([128, 8, 112], MDT, tag="m1")
                nc.vector.tensor_mul(m1[:], y16[:], mt["s1f"][:, r0:r0 + 8, :])
                g = ep.tile([128, 8, 112], MDT, tag="g")
                nc.scalar.activation(g[:], m1[:], SG)
                m2 = ep.tile([128, 8, 112], MDT, tag="m2")
                nc.vector.tensor_mul(m2[:], y16[:], mt["bf"][:, r0:r0 + 8, :])
                nc.vector.tensor_add(m2[:], m2[:], mt["af"][:, r0:r0 + 8, :])
                nc.vector.tensor_mul(ht[:, r0 + 1:r0 + 9, 1:113], g[:], m2[:])
            # halo exchange between the two halves of ht
            nc.gpsimd.dma_start(ht[0:64, HP - 1, 1:113], ht[64:128, 1, 1:113])
            nc.gpsimd.dma_start(ht[64:128, 0, 1:113], ht[0:64, SEC, 1:113])

        def sc_chain(n, u, ve=None):
            """shortcut act for unit u -> sv tile (depends only on x + masks)"""
            r0 = 8 * u
            xi = xts[n][:, r0 + 1:r0 + 9, 1:113]
            t1 = ep.tile([128, 8, 112], MDT, tag="t1", name=f"t1_{n}_{u}")
            nc.vector.tensor_mul(t1[:], xi, mt["s1s"][:, r0:r0 + 8, :])
            gs = ep.tile([128, 8, 112], MDT, tag="gs", name=f"gs_{n}_{u}")
            nc.scalar.activation(gs[:], t1[:], SG)
            t2 = ep.tile([128, 8, 112], MDT, tag="t2", name=f"t2_{n}_{u}")
            ve = ve or nc.gpsimd
            ve.tensor_mul(t2[:], xi, mt["bs"][:, r0:r0 + 8, :])
            ve.tensor_add(t2[:], t2[:], mt["as_"][:, r0:r0 + 8, :])
            sv = ep.tile([128, 8, 112], MDT, tag="sv", name=f"sv_{n}_{u}")
            nc.vector.tensor_mul(sv[:], gs[:], t2[:])
            # fold the constant map in here, off the post-matmul critical path
            nc.vector.tensor_add(sv[:], sv[:], mt["kmap"][:, r0:r0 + 8, :])
            return sv[:]

        def l2_phase(n):
            ht = hts[n]
            last = n == BPC - 1
            for u in range(NU):
                r0 = 8 * u
                ps = pp.tile([128, 8, 128], mybir.dt.float32, tag="ps2")
                conv_unit(ht, w2t, ps, r0)
                sv = sc_chain(n, u, ve=nc.vector if last else None)
                o1 = ep.tile([128, 8, 112], MDT, tag="o1", bufs=3)
                if last and u == NU - 1:
                    # split the combine per 4-row sub-block so the tail DMA
                    # starts as soon as the final matmul group lands
                    for i in (0, 1):
                        rows = slice(4 * i, 4 * i + 4)
                        nc.vector.tensor_add(
                            o1[:, rows, :], ps[:, rows, 0:112], sv[:, rows, :])
                        nc.sync.dma_start(
                            outd[n, :, r0 + 4 * i:r0 + 4 * i + 4, :], o1[:, rows, :])
                else:
                    nc.vector.tensor_add(o1[:], ps[:, :, 0:112], sv)
                    nc.sync.dma_start(outd[n, :, r0:r0 + 8, :], o1[:])

        for n in range(BPC):
            l1_phase(n)
            if n >= 1:
                l2_phase(n - 1)
        l2_phase(BPC - 1)

    nc.compile()
    return nc


def kernel(x, conv1_w, conv2_w, gamma1, beta1, mean1, var1,
           gamma2, beta2, mean2, var2, act_codes_feat, act_codes_sc):
    x = np.asarray(x, np.float32)
    a1 = (np.asarray(gamma1) / np.sqrt(np.asarray(var1) + EPS)).astype(np.float32)
    b1 = (np.asarray(beta1) - np.asarray(mean1) * a1).astype(np.float32)
    a2 = (np.asarray(gamma2) / np.sqrt(np.asarray(var2) + EPS)).astype(np.float32)
    b2 = (np.asarray(beta2) - np.asarray(mean2) * a2).astype(np.float32)

    s1f, bf, af, d1 = _mask_arrays(np.asarray(act_codes_feat))
    s1s, bs, as_, d2 = _mask_arrays(np.asarray(act_codes_sc))

    # fold BN scales into conv weights (per output channel)
    w1f = np.asarray(conv1_w, np.float32) * a1[:, None, None, None]
    w2f = np.asarray(conv2_w, np.float32) * a2[:, None, None, None]

    # K = conv2_bnfolded(d1map) + bn2 bias + d2map  (all constant)
    kmap = _host_conv3x3(d1, w2f) + b2[:, None, None] + d2

    # block-diagonal stationary weights, SBUF-major [128(k=in), 18(tap), 128(m=out)]
    wh = np.zeros((128, 18, 128), F16)
    for t, (ky, kx) in enumerate(TAPS):
        wh[0:64, t, 0:64] = wh[64:128, t, 64:128] = \
            w1f[:, :, ky + 1, kx + 1].T.astype(F16)
        wh[0:64, 9 + t, 0:64] = wh[64:128, 9 + t, 64:128] = \
            w2f[:, :, ky + 1, kx + 1].T.astype(F16)

    b1h = np.concatenate([b1, b1]).reshape(128, 1).astype(np.float32)

    nc = _build_program()

    marrs = {
        "s1f": s1f, "bf": bf, "af": af,
        "s1s": s1s, "bs": bs, "as_": as_, "kmap": kmap,
    }
    marrs = {k: _split_halves(v).astype(F16) for k, v in marrs.items()}

    in_maps = []
    for core in range(NCORES):
        xs = np.stack([
            _pad_split_image(x[core * BPC + i]) for i in range(BPC)
        ])
        im = {"xin": xs, "w": wh, "b1": b1h}
        im.update(marrs)
        in_maps.append(im)

    res = run_bass_kernel_spmd(nc, in_maps, core_ids=list(range(NCORES)))
    global LAST_RESULT
    LAST_RESULT = res

    out = np.empty((B, C, H, W), np.float32)
    for core in range(NCORES):
        o = res.results[core]["out"]  # [BPC, 128, 56, 112] f16
        for i in range(BPC):
            img = np.concatenate([o[i, 0:64], o[i, 64:128]], axis=1)
            out[core * BPC + i] = img.astype(np.float32)
    return out


if __name__ == "__main__":
    rng = np.random.default_rng(0)
    inputs = {
        "x": rng.standard_normal((B, C, H, W)).astype(np.float32),
        "conv1_w": (rng.standard_normal((C, C, 3, 3)) * 0.05).astype(np.float32),
        "conv2_w": (rng.standard_normal((C, C, 3, 3)) * 0.05).astype(np.float32),
        "gamma1": np.ones(C, np.float32), "beta1": np.zeros(C, np.float32),
        "mean1": np.zeros(C, np.float32), "var1": np.ones(C, np.float32),
        "gamma2": np.ones(C, np.float32), "beta2": np.zeros(C, np.float32),
        "mean2": np.zeros(C, np.float32), "var2": np.ones(C, np.float32),
        "act_codes_feat": rng.integers(0, 4, C * H * W).astype(np.int32),
        "act_codes_sc": rng.integers(0, 4, C * H * W).astype(np.int32),
    }
    out = kernel(**inputs)
    print("out", out.shape, out.dtype, float(np.abs(out).max()))
